# revision 1
# baseline (speedup 1.0000x reference)
"""ChoiceAttention Trainium2 kernel (fp16 rewrite).

Math (per batch item, per option a):
    q_a = opt_a @ W                           (s, h)
    S_ak[i, j] = q_a[i, :] . opt_k[j, :]      for the 4 options k != a
    w_ak = softmax_k(S_ak)                    (bias cancels - shift invariant)
    out += sum_k (sum_a w_ak) @ opt_k         (wsum collapse: 4x fewer matmuls)
final out /= 2.

Sharding: data-parallel over batch across 8 NeuronCores (4 items each).

Key design vs the fp32r baseline:
  - all matmuls fp16 (full PE rate + fast weight load; L2 err ~8e-4 vs 2e-2 gate)
  - options are transposed + converted to fp16 on the HOST: both natural (nat)
    and h-major (xt) layouts are DMA'd, eliminating all PE transposes
  - q batched over all 5 options with W stationary (few weight loads)
  - scores k-stationary -> softmax yields w^T directly for the out matmul
  - PE phase order per item: scores(b) -> q(b+1) -> out(b), so the softmax of
    item b (DVE/ACT/GPSIMD) hides under q(b+1) matmuls
  - softmax all-fp16 in SBUF (2x DVE mode), spread across DVE/ACT/GPSIMD

Layouts per core / item (P=128 partitions):
    XT  [P, hc=8, a=5, s=256]  fp16   opt^T, h-major (h = hc*128 + p)
    NAT [P, k=5, jb=2, h=1024] fp16   opt natural    (j = jb*128 + p)
    W   [P, kc=8, m=1024]      fp16   (k = kc*128 + p)
    Q   [P, hc=8, a=5, i=256]  fp16   q^T, h-major
    ST[k] [P, jb=2, slot=4, i=256] fp16  S^T (j on partitions), slot = a<k?a:a-1
    WS[k] [P, jb=2, i=256]     fp16   wsum^T
    out psum [i-part, h]; OSB [P, ib=2, h=1024] fp16 -> DMA (host upcasts)
"""

import numpy as np

B, S, H = 32, 256, 1024
NCORES = 8
BPC = B // NCORES
P = 128
HC = H // P   # 8
SC = S // P   # 2 (jb / ib chunks)
NOPT = 5

# scores psum segments per k: (psum_col0//256, a0, n_a) with a != k packed
SEGS = {
    0: [(0, 1, 2), (2, 3, 2)],
    1: [(0, 0, 1), (1, 2, 1), (2, 3, 2)],
    2: [(0, 0, 2), (2, 3, 2)],
    3: [(0, 0, 2), (2, 2, 1), (3, 4, 1)],
    4: [(0, 0, 2), (2, 2, 2)],
}

_CACHE: dict = {}


def _build_bass(reps: int = 1, cfg: dict | None = None):
    cfg = dict(cfg or {})
    GP_SUB = cfg.get("gp_sub", False)
    GP_MUL = cfg.get("gp_mul", False)
    from contextlib import ExitStack, nullcontext

    import concourse.mybir as mybir
    import concourse.tile as tile
    from concourse import bacc

    FP32 = mybir.dt.float32
    F16 = mybir.dt.float16
    AF = mybir.ActivationFunctionType

    nc = bacc.Bacc(debug=False)

    xt_d = [nc.dram_tensor(f"xt{i + 1}", (BPC, H, S), F16, kind="ExternalInput")
            for i in range(NOPT)]
    nat_d = [nc.dram_tensor(f"nat{i + 1}", (BPC, S, H), F16, kind="ExternalInput")
             for i in range(NOPT)]
    w_d = nc.dram_tensor("W", (H, H), F16, kind="ExternalInput")
    out_d = nc.dram_tensor("out", (BPC, S, H), F16, kind="ExternalOutput")

    with ExitStack() as ctx:
        tc = ctx.enter_context(tile.TileContext(nc))
        const = ctx.enter_context(tc.tile_pool(name="const", bufs=1))
        xtp = ctx.enter_context(tc.tile_pool(name="xt", bufs=2))
        natp = ctx.enter_context(tc.tile_pool(name="nat", bufs=3))
        qp = ctx.enter_context(tc.tile_pool(name="qq", bufs=1))
        stp = ctx.enter_context(tc.tile_pool(name="st", bufs=NOPT))
        ep = ctx.enter_context(tc.tile_pool(name="ee", bufs=2))
        ttp = ctx.enter_context(tc.tile_pool(name="tt", bufs=4))
        mp_ = ctx.enter_context(tc.tile_pool(name="mm", bufs=2))
        zp = ctx.enter_context(tc.tile_pool(name="zz", bufs=2))
        rp = ctx.enter_context(tc.tile_pool(name="rr", bufs=2))
        wsp = ctx.enter_context(tc.tile_pool(name="ws", bufs=2 * NOPT))
        op_ = ctx.enter_context(tc.tile_pool(name="osb", bufs=2))
        ps_big = ctx.enter_context(tc.tile_pool(name="ps_big", bufs=2, space="PSUM"))
        ps_sm = ctx.enter_context(tc.tile_pool(name="ps_sm", bufs=1, space="PSUM"))
        ps_o = ctx.enter_context(tc.tile_pool(name="ps_o", bufs=2, space="PSUM"))

        w_sb = const.tile([P, HC, H], F16)

        loop_cm = tc.For_i(0, reps, 1) if reps > 1 else nullcontext()
        with loop_cm:
            xts = [None] * BPC
            nats = [None] * BPC
            qs = [None] * BPC

            def load_item(b):
                xts[b] = xtp.tile([P, HC, NOPT, S], F16, tag="xt", name=f"xt_{b}")
                nats[b] = natp.tile([P, NOPT, SC, H], F16, tag="nat", name=f"nat_{b}")
                # xt first: q(b) starts as soon as xt lands; nat only needed
                # at out(b)
                for kk in range(NOPT):
                    nc.sync.dma_start(
                        out=xts[b][:, :, kk, :],
                        in_=xt_d[kk].ap()[b].rearrange("(hc p) s -> p hc s", p=P),
                    )
                for kk in range(NOPT):
                    nc.sync.dma_start(
                        out=nats[b][:, kk, :, :],
                        in_=nat_d[kk].ap()[b].rearrange("(sc p) h -> p sc h", p=P),
                    )

            # round-robin over [ACT, DVE, GPSIMD] for psum-evac copies
            ev_state = [0]

            def evac(out_ap, in_ap):
                # GPSIMD cannot access PSUM: alternate ACT / DVE
                i = ev_state[0] % 2
                ev_state[0] += 1
                if i == 0:
                    nc.scalar.copy(out=out_ap, in_=in_ap)
                else:
                    nc.vector.tensor_copy(out=out_ap, in_=in_ap)

            def emit_q(b):
                """Q(b) = q^T for all 5 options, W stationary, a-batched."""
                q = qp.tile([P, HC, NOPT, S], F16, tag="qq", name=f"q_{b}")
                for mc in range(HC):
                    qA = ps_big.tile([P, 4, S], FP32, tag="big", name=f"qA_{b}_{mc}")
                    qB = ps_sm.tile([P, S], FP32, tag="sm", name=f"qB_{b}_{mc}")
                    for kc in range(HC):
                        lhsT = w_sb[:, kc, mc * P:(mc + 1) * P]
                        st_, sp_ = (kc == 0), (kc == HC - 1)
                        nc.tensor.matmul(qA[:, 0:2, :], lhsT, xts[b][:, kc, 0:2, :],
                                         start=st_, stop=sp_)
                        nc.tensor.matmul(qA[:, 2:4, :], lhsT, xts[b][:, kc, 2:4, :],
                                         start=st_, stop=sp_)
                        nc.tensor.matmul(qB, lhsT, xts[b][:, kc, 4, :],
                                         start=st_, stop=sp_)
                    evac(q[:, mc, 0:4, :], qA)
                    evac(q[:, mc, 4, :], qB)
                qs[b] = q

            def emit_scores(b):
                """ST[k] = S^T for all a != k; k-stationary over xt chunks."""
                sts = []
                for k in range(NOPT):
                    stt = stp.tile([P, SC, 4, S], F16, tag="st", name=f"st_{b}_{k}")
                    for jb in range(SC):
                        pb = ps_big.tile([P, 4, S], FP32, tag="big",
                                         name=f"ps_{b}_{k}_{jb}")
                        # one psum accumulation group per bank: start only on
                        # the first seg of each bank, stop on the last
                        first_in_bank = {}
                        last_in_bank = {}
                        for si, (c0, a0, na) in enumerate(SEGS[k]):
                            bank = c0 // 2
                            first_in_bank.setdefault(bank, si)
                            last_in_bank[bank] = si
                        for hc in range(HC):
                            lhsT = xts[b][:, hc, k, jb * P:(jb + 1) * P]
                            for si, (c0, a0, na) in enumerate(SEGS[k]):
                                bank = c0 // 2
                                nc.tensor.matmul(
                                    pb[:, c0:c0 + na, :], lhsT,
                                    qs[b][:, hc, a0:a0 + na, :],
                                    start=(hc == 0 and first_in_bank[bank] == si),
                                    stop=(hc == HC - 1 and last_in_bank[bank] == si))
                        evac(stt[:, jb, :, :], pb)
                    sts.append(stt)
                return sts

            def emit_softmax(b, sts, ws):
                for a in range(NOPT):
                    ks = [k for k in range(NOPT) if k != a]
                    sl = [sts[k][:, :, a - (1 if a > k else 0), :] for k in ks]
                    t0 = ttp.tile([P, SC, S], F16, tag="tt", name=f"t0_{b}_{a}")
                    t1 = ttp.tile([P, SC, S], F16, tag="tt", name=f"t1_{b}_{a}")
                    m = mp_.tile([P, SC, S], F16, tag="mm", name=f"m_{b}_{a}")
                    nc.vector.tensor_max(t0, sl[0], sl[1])
                    nc.vector.tensor_max(t1, sl[2], sl[3])
                    nc.vector.tensor_max(m, t0, t1)
                    e = ep.tile([P, SC, 4, S], F16, tag="ee", name=f"e_{b}_{a}")
                    for k4 in range(4):
                        eng = nc.gpsimd if (GP_SUB and k4 < 2) else nc.vector
                        eng.tensor_sub(e[:, :, k4, :], sl[k4], m)
                    nc.scalar.activation(out=e, in_=e, func=AF.Exp)
                    z = zp.tile([P, SC, S], F16, tag="zz", name=f"z_{b}_{a}")
                    nc.vector.tensor_add(t0, e[:, :, 0, :], e[:, :, 1, :])
                    nc.vector.tensor_add(t1, e[:, :, 2, :], e[:, :, 3, :])
                    nc.vector.tensor_add(z, t0, t1)
                    r = rp.tile([P, SC, S], FP32, tag="rr", name=f"r_{b}_{a}")
                    nc.vector.reciprocal(r, z)
                    for k4, k in enumerate(ks):
                        if ws[k] is None:
                            ws[k] = wsp.tile([P, SC, S], F16, tag="ws",
                                             name=f"ws_{b}_{k}")
                            nc.vector.tensor_mul(ws[k], e[:, :, k4, :], r)
                        else:
                            nc.vector.tensor_mul(e[:, :, k4, :], e[:, :, k4, :], r)
                            eng = nc.gpsimd if (GP_MUL and k4 % 2) else nc.vector
                            eng.tensor_add(ws[k], ws[k], e[:, :, k4, :])

            def emit_out(b, ws):
                osb = op_.tile([P, SC, H], F16, tag="osb", name=f"osb_{b}")
                for ib in range(SC):
                    for hb in range(2):
                        po = ps_o.tile([P, 512], FP32, tag="o",
                                       name=f"po_{b}_{ib}_{hb}")
                        n = 0
                        # k=4 first: ws[4] completes one softmax iteration
                        # before the rest, letting the tail chains start early
                        for k in (4, 0, 1, 2, 3):
                            for jb in range(SC):
                                nc.tensor.matmul(
                                    po, ws[k][:, jb, ib * P:(ib + 1) * P],
                                    nats[b][:, k, jb, hb * 512:(hb + 1) * 512],
                                    start=(n == 0), stop=(n == 2 * NOPT - 1))
                                n += 1
                        nc.scalar.activation(out=osb[:, ib, hb * 512:(hb + 1) * 512],
                                             in_=po, func=AF.Copy, scale=0.5)
                nc.scalar.dma_start(
                    out=out_d.ap()[b].rearrange("(sc p) h -> p sc h", p=P), in_=osb)

            # ---- schedule ----
            # PE order: s0 q1 | s1 q2 o0 | s2 q3 o1 | s3 o2 | o3 — each out
            # is delayed one stage so the last softmax hides under out(2)
            load_item(0)
            wap = w_d.ap().rearrange("(kc p) m -> p kc m", p=P)
            for kc in range(HC):
                nc.scalar.dma_start(out=w_sb[:, kc, :], in_=wap[:, kc, :])
            if BPC > 1:
                load_item(1)
            emit_q(0)
            ws_all = []
            for b in range(BPC):
                sts = emit_scores(b)
                if b + 1 < BPC:
                    emit_q(b + 1)
                ws = [None] * NOPT
                emit_softmax(b, sts, ws)
                ws_all.append(ws)
                if b >= 1:
                    emit_out(b - 1, ws_all[b - 1])
                if b + 2 < BPC:
                    # after out(b-1): its nat slot may be recycled by this DMA
                    load_item(b + 2)
            emit_out(BPC - 1, ws_all[BPC - 1])

    nc.compile()
    return nc


def _get_nc(reps: int = 1, cfg: dict | None = None):
    key = f"nc{reps}-{sorted((cfg or {}).items())}"
    if key not in _CACHE:
        _CACHE[key] = _build_bass(reps, cfg)
    return _CACHE[key]


def _prep(inputs):
    opts = [np.asarray(inputs[f"option{i + 1}"], dtype=np.float32)
            for i in range(NOPT)]
    nat = [np.ascontiguousarray(o.astype(np.float16)) for o in opts]
    xt = [np.ascontiguousarray(o.transpose(0, 2, 1).astype(np.float16))
          for o in opts]
    W = np.ascontiguousarray(np.asarray(inputs["W"], np.float32).astype(np.float16))
    return nat, xt, W


def kernel(**inputs) -> np.ndarray:
    from concourse.bass_utils import run_bass_kernel_spmd

    nc = _get_nc()
    nat, xt, W = _prep(inputs)

    in_maps = []
    for c in range(NCORES):
        m = {}
        for i in range(NOPT):
            m[f"xt{i + 1}"] = xt[i][c * BPC:(c + 1) * BPC]
            m[f"nat{i + 1}"] = nat[i][c * BPC:(c + 1) * BPC]
        m["W"] = W
        in_maps.append(m)

    res = run_bass_kernel_spmd(nc, in_maps, list(range(NCORES)))
    out = np.concatenate([res.results[c]["out"] for c in range(NCORES)], axis=0)
    return np.asarray(out, dtype=np.float32)



# revision 2
# speedup vs baseline: 1.0928x; 1.0928x over previous
"""ChoiceAttention Trainium2 kernel v2.

vs v1 (448 us HW):
  - scores land in ONE tile ST[P, k=5, jb=2, a=5, i=256] (diagonal a==k slots
    memset to -30000 by gpsimd): softmax batches elementwise ops over all a
    and k (k via stride-0 broadcast), cutting DVE instrs/item from ~85 to ~26.
  - softmax stages pipelined per jb (sub/exp/z/recip/mul/ws) so ACT exp
    overlaps DVE work.
  - item 3 (the tail) computes scores+softmax per i-half: softmax half 1
    hides under out(1)+out(2), out(3).ib0 needs only half 0.
  - dedicated 1-bank psum pool for out groups + item-3 score groups: out
    matmuls no longer wait on scores psum rotation.
  - W DMA split per-kc, interleaved into xt(0) option DMAs; nat deferred.
"""

import numpy as np

B, S, H = 32, 256, 1024
NCORES = 8
BPC = B // NCORES
P = 128
HC = H // P   # 8
SC = S // P   # 2
NOPT = 5

# scores psum layout per k: columns = a ascending, skipping a==k
SEGS = {
    0: [(0, 1, 2), (2, 3, 2)],
    1: [(0, 0, 1), (1, 2, 1), (2, 3, 2)],
    2: [(0, 0, 2), (2, 3, 2)],
    3: [(0, 0, 2), (2, 2, 1), (3, 4, 1)],
    4: [(0, 0, 2), (2, 2, 2)],
}

NEG = -30000.0

_CACHE: dict = {}


def _build_bass(reps: int = 1, cfg: dict | None = None):
    cfg = dict(cfg or {})
    from contextlib import ExitStack, nullcontext

    import concourse.mybir as mybir
    import concourse.tile as tile
    from concourse import bacc

    FP32 = mybir.dt.float32
    F16 = mybir.dt.float16
    AF = mybir.ActivationFunctionType

    nc = bacc.Bacc(debug=False)

    xt_d = [nc.dram_tensor(f"xt{i + 1}", (BPC, H, S), F16, kind="ExternalInput")
            for i in range(NOPT)]
    nat_d = [nc.dram_tensor(f"nat{i + 1}", (BPC, S, H), F16, kind="ExternalInput")
             for i in range(NOPT)]
    w_d = nc.dram_tensor("W", (H, H), F16, kind="ExternalInput")
    out_d = nc.dram_tensor("out", (BPC, S, H), F16, kind="ExternalOutput")

    with ExitStack() as ctx:
        tc = ctx.enter_context(tile.TileContext(nc))
        const = ctx.enter_context(tc.tile_pool(name="const", bufs=1))
        xtp = ctx.enter_context(tc.tile_pool(name="xt", bufs=2))
        natp = ctx.enter_context(tc.tile_pool(name="nat", bufs=2))
        qp = ctx.enter_context(tc.tile_pool(name="qq", bufs=1))
        stp = ctx.enter_context(tc.tile_pool(name="st", bufs=2))
        mzp = ctx.enter_context(tc.tile_pool(name="mz", bufs=2))
        rp = ctx.enter_context(tc.tile_pool(name="rr", bufs=1))
        wsp = ctx.enter_context(tc.tile_pool(name="ws", bufs=3))
        op_ = ctx.enter_context(tc.tile_pool(name="osb", bufs=2))
        ps_big = ctx.enter_context(tc.tile_pool(name="ps_big", bufs=2, space="PSUM"))
        ps_sm = ctx.enter_context(tc.tile_pool(name="ps_sm", bufs=1, space="PSUM"))
        ps_o = ctx.enter_context(tc.tile_pool(name="ps_o", bufs=3, space="PSUM"))

        w_sb = const.tile([P, HC, H], F16)
        wap = w_d.ap().rearrange("(kc p) m -> p kc m", p=P)

        loop_cm = tc.For_i(0, reps, 1) if reps > 1 else nullcontext()
        with loop_cm:
            xts = [None] * BPC
            nats = [None] * BPC
            qs = [None] * BPC

            def xt_dma(b, kk):
                nc.sync.dma_start(
                    out=xts[b][:, :, kk, :],
                    in_=xt_d[kk].ap()[b].rearrange("(hc p) s -> p hc s", p=P))

            def load_xt(b, first=False):
                xts[b] = xtp.tile([P, HC, NOPT, S], F16, tag="xt", name=f"xt_{b}")
                if first:
                    # interleave: q(0) pass 1 starts after opt0.h0, opt1.h0, W0
                    def wc(kc):
                        nc.scalar.dma_start(out=w_sb[:, kc, :], in_=wap[:, kc, :])

                    def xt_half(kk, h):
                        hs = slice(h * 4, (h + 1) * 4)
                        nc.sync.dma_start(
                            out=xts[b][:, hs, kk, :],
                            in_=xt_d[kk].ap()[b].rearrange(
                                "(hc p) s -> p hc s", p=P)[:, hs, :])
                    xt_half(0, 0)
                    xt_half(1, 0)
                    wc(0)
                    wc(1)
                    xt_half(0, 1)
                    xt_half(1, 1)
                    wc(2)
                    wc(3)
                    xt_dma(b, 2)
                    wc(4)
                    wc(5)
                    xt_dma(b, 3)
                    for kc in range(6, HC):
                        wc(kc)
                    xt_dma(b, 4)
                else:
                    for kk in range(NOPT):
                        xt_dma(b, kk)

            def load_nat(b):
                nats[b] = natp.tile([P, NOPT, SC, H], F16, tag="nat",
                                    name=f"nat_{b}")
                for kk in range(NOPT):
                    nc.sync.dma_start(
                        out=nats[b][:, kk, :, :],
                        in_=nat_d[kk].ap()[b].rearrange("(sc p) h -> p sc h", p=P))

            ev_state = [0]

            def evac(out_ap, in_ap, eng=None, scale=None):
                if eng is None:
                    eng = ("act", "dve")[ev_state[0] % 2]
                    ev_state[0] += 1
                if scale is None:
                    if eng == "act":
                        nc.scalar.copy(out=out_ap, in_=in_ap)
                    else:
                        nc.vector.tensor_copy(out=out_ap, in_=in_ap)
                else:
                    if eng == "act":
                        nc.scalar.activation(out=out_ap, in_=in_ap, func=AF.Copy,
                                             scale=scale)
                    else:
                        nc.vector.tensor_scalar_mul(out_ap, in_ap, scale)

            def emit_q(b):
                """Q(b) = q^T for all 5 options, W stationary, a-batched."""
                q = qp.tile([P, HC, NOPT, S], F16, tag="qq", name=f"q_{b}")
                for mc in range(HC):
                    qA = ps_big.tile([P, 4, S], FP32, tag="big", name=f"qA_{b}_{mc}")
                    qB = ps_sm.tile([P, S], FP32, tag="sm", name=f"qB_{b}_{mc}")
                    for kc in range(HC):
                        lhsT = w_sb[:, kc, mc * P:(mc + 1) * P]
                        st_, sp_ = (kc == 0), (kc == HC - 1)
                        nc.tensor.matmul(qA[:, 0:2, :], lhsT, xts[b][:, kc, 0:2, :],
                                         start=st_, stop=sp_)
                        nc.tensor.matmul(qA[:, 2:4, :], lhsT, xts[b][:, kc, 2:4, :],
                                         start=st_, stop=sp_)
                        nc.tensor.matmul(qB, lhsT, xts[b][:, kc, 4, :],
                                         start=st_, stop=sp_)
                    evac(q[:, mc, 0:4, :], qA)
                    evac(q[:, mc, 4, :], qB)
                qs[b] = q

            def emit_q0_split(b):
                """item-0 q in two passes. pass 1 (a0-1) is kc-OUTER with all
                8 mc psum groups open (uses the whole psum): matmuls start
                after just opt0+opt1+W0 and pace with the W chunk arrivals."""
                q = qp.tile([P, HC, NOPT, S], F16, tag="qq", name=f"q_{b}")
                grp = []
                for mc in range(4):
                    if mc % 2 == 0:
                        t = ps_big.tile([P, 4, S], FP32, tag="big",
                                        name=f"q0a_{mc}")
                        grp.append(t[:, 0:2, :])
                        grp.append(t[:, 2:4, :])
                for mc in range(4, 7):
                    t = ps_o.tile([P, 4, P], FP32, tag="o", name=f"q0a_{mc}")
                    grp.append(t.rearrange("p a i -> p (a i)")
                               .rearrange("p (a s) -> p a s", s=S))
                t = ps_sm.tile([P, 2, S], FP32, tag="sm", name="q0a_7")
                grp.append(t)
                for kc in range(HC):
                    for mc in range(HC):
                        nc.tensor.matmul(grp[mc],
                                         w_sb[:, kc, mc * P:(mc + 1) * P],
                                         xts[b][:, kc, 0:2, :],
                                         start=(kc == 0), stop=(kc == HC - 1))
                for mc in range(HC):
                    evac(q[:, mc, 0:2, :], grp[mc])
                for mc in range(HC):
                    qA = ps_big.tile([P, 4, S], FP32, tag="big",
                                     name=f"q0b_{mc}")
                    qB = ps_sm.tile([P, S], FP32, tag="sm", name=f"q0c_{mc}")
                    for kc in range(HC):
                        lhsT = w_sb[:, kc, mc * P:(mc + 1) * P]
                        st_, sp_ = (kc == 0), (kc == HC - 1)
                        nc.tensor.matmul(qA[:, 2:4, :], lhsT, xts[b][:, kc, 2:4, :],
                                         start=st_, stop=sp_)
                        nc.tensor.matmul(qB, lhsT, xts[b][:, kc, 4, :],
                                         start=st_, stop=sp_)
                    evac(q[:, mc, 2:4, :], qA[:, 2:4, :])
                    evac(q[:, mc, 4, :], qB)
                qs[b] = q

            def st_alloc(b):
                st = stp.tile([P, NOPT, SC, NOPT, S], F16, tag="st", name=f"st_{b}")
                for k in range(NOPT):
                    nc.gpsimd.memset(st[:, k, :, k, :], NEG)
                return st

            def emit_scores(b, st):
                """full-i scores: psum [P,4,S] groups per (k,jb)."""
                for k in range(NOPT):
                    for jb in range(SC):
                        pb = ps_big.tile([P, 4, S], FP32, tag="big",
                                         name=f"ps_{b}_{k}_{jb}")
                        first_in_bank = {}
                        last_in_bank = {}
                        for si, (c0, a0, na) in enumerate(SEGS[k]):
                            bank = c0 // 2
                            first_in_bank.setdefault(bank, si)
                            last_in_bank[bank] = si
                        for hc in range(HC):
                            lhsT = xts[b][:, hc, k, jb * P:(jb + 1) * P]
                            for si, (c0, a0, na) in enumerate(SEGS[k]):
                                bank = c0 // 2
                                nc.tensor.matmul(
                                    pb[:, c0:c0 + na, :], lhsT,
                                    qs[b][:, hc, a0:a0 + na, :],
                                    start=(hc == 0 and first_in_bank[bank] == si),
                                    stop=(hc == HC - 1 and last_in_bank[bank] == si))
                        if k > 0:
                            evac(st[:, k, jb, 0:k, :], pb[:, 0:k, :])
                        if k < NOPT - 1:
                            evac(st[:, k, jb, k + 1:NOPT, :], pb[:, k:4, :])

            def emit_scores_half(b, st, ih):
                """i-half scores for the tail item: 1-bank psum per (k,jb)."""
                isl = slice(ih * P, (ih + 1) * P)
                for k in range(NOPT):
                    for jb in range(SC):
                        # alternate the two psum pools: 5 groups in flight so
                        # a briefly blocked ACT evac queue can't stall the PE
                        if (k * SC + jb) % 2 == 0:
                            pb = ps_o.tile([P, 4, P], FP32, tag="o",
                                           name=f"ph_{b}_{k}_{jb}_{ih}")
                        else:
                            pb = ps_big.tile([P, 4, P], FP32, tag="big",
                                             name=f"ph_{b}_{k}_{jb}_{ih}")
                        nseg = len(SEGS[k])
                        for hc in range(HC):
                            lhsT = xts[b][:, hc, k, jb * P:(jb + 1) * P]
                            for si, (c0, a0, na) in enumerate(SEGS[k]):
                                nc.tensor.matmul(
                                    pb[:, c0:c0 + na, :], lhsT,
                                    qs[b][:, hc, a0:a0 + na, isl],
                                    start=(hc == 0 and si == 0),
                                    stop=(hc == HC - 1 and si == nseg - 1))
                        if k > 0:
                            evac(st[:, k, jb, 0:k, isl], pb[:, 0:k, :], eng="act")
                        if k < NOPT - 1:
                            evac(st[:, k, jb, k + 1:NOPT, isl], pb[:, k:4, :],
                                 eng="act")

            def emit_softmax(b, st, ws, cs=slice(0, S)):
                """softmax over k, batched over (jb, a, i in cs); jb-pipelined."""
                n = cs.stop - cs.start
                m = mzp.tile([P, SC, NOPT, S], F16, tag="mz", name=f"m_{b}_{cs.start}")
                z = mzp.tile([P, SC, NOPT, S], F16, tag="mz", name=f"z_{b}_{cs.start}")
                r = rp.tile([P, SC, NOPT, S], F16, tag="rr", name=f"r_{b}_{cs.start}")
                mc = m[:, :, :, cs]
                nc.vector.tensor_max(mc, st[:, 0, :, :, cs], st[:, 1, :, :, cs])
                for k in range(2, NOPT):
                    nc.vector.tensor_max(mc, mc, st[:, k, :, :, cs])
                for jb in range(SC):
                    stj = st[:, :, jb, :, cs]          # [P, 5k, 5a, n]
                    mb = m[:, jb, :, cs].unsqueeze(1).broadcast_to(
                        (P, NOPT, NOPT, n))
                    nc.vector.tensor_sub(stj, stj, mb)
                    nc.scalar.activation(out=stj, in_=stj, func=AF.Exp)
                    zj = z[:, jb, :, cs]
                    nc.vector.tensor_add(zj, st[:, 0, jb, :, cs],
                                         st[:, 1, jb, :, cs])
                    for k in range(2, NOPT):
                        nc.vector.tensor_add(zj, zj, st[:, k, jb, :, cs])
                    rj = r[:, jb, :, cs]
                    with nc.allow_low_precision(reason="1/z f16: w err ~5e-4"):
                        nc.vector.reciprocal(rj, zj)
                    rb = rj.unsqueeze(1).broadcast_to((P, NOPT, NOPT, n))
                    nc.vector.tensor_mul(stj, stj, rb)
                    # ws[k] = sum_a w[k, a]; diagonal contributes 0
                    nc.vector.tensor_add(st[:, :, jb, 0, cs], st[:, :, jb, 0, cs],
                                         st[:, :, jb, 1, cs])
                    nc.vector.tensor_add(st[:, :, jb, 2, cs], st[:, :, jb, 2, cs],
                                         st[:, :, jb, 3, cs])
                    nc.vector.tensor_add(st[:, :, jb, 0, cs], st[:, :, jb, 0, cs],
                                         st[:, :, jb, 2, cs])
                    nc.vector.tensor_add(ws[:, :, jb, cs], st[:, :, jb, 0, cs],
                                         st[:, :, jb, 4, cs])

            def emit_out(b, ws, ibs=(0, 1)):
                osb = op_.tile([P, SC, H], F16, tag="osb", name=f"osb_{b}")
                oap = out_d.ap()[b].rearrange("(sc p) h -> p sc h", p=P)
                for ib in ibs:
                    pos = [ps_o.tile([P, 512], FP32, tag="o",
                                     name=f"po_{b}_{ib}_{hb}") for hb in range(2)]
                    n = 0
                    for k in range(NOPT):
                        for jb in range(SC):
                            lhsT = ws[:, k, jb, ib * P:(ib + 1) * P]
                            st_, sp_ = (n == 0), (n == 2 * NOPT - 1)
                            for hb in range(2):
                                nc.tensor.matmul(pos[hb], lhsT,
                                                 nats[b][:, k, jb,
                                                         hb * 512:(hb + 1) * 512],
                                                 start=st_, stop=sp_)
                            n += 1
                    for hb in range(2):
                        evac(osb[:, ib, hb * 512:(hb + 1) * 512], pos[hb],
                             scale=0.5)
                    nc.scalar.dma_start(out=oap[:, ib, :], in_=osb[:, ib, :])

            # ---- schedule (PE order):
            # q0 s0 q1 s1 q2 o0 s2 q3 s3.h0 s3.h1 o1 o2 o3.ib0 o3.ib1
            load_xt(0, first=True)
            load_nat(0)
            load_xt(1)
            emit_q0_split(0)

            def ws_alloc(b):
                return wsp.tile([P, NOPT, SC, S], F16, tag="ws", name=f"ws_{b}")

            ws_all = [None] * BPC
            st0 = st_alloc(0)
            emit_scores(0, st0)
            load_nat(1)
            emit_q(1)
            ws_all[0] = ws_alloc(0)
            emit_softmax(0, st0, ws_all[0])
            st1 = st_alloc(1)
            emit_scores(1, st1)
            load_xt(2)
            emit_q(2)
            ws_all[1] = ws_alloc(1)
            emit_softmax(1, st1, ws_all[1])
            emit_out(0, ws_all[0])
            load_xt(3)
            st2 = st_alloc(2)
            emit_scores(2, st2)
            load_nat(2)
            emit_q(3)
            st3 = st_alloc(3)
            ws_all[3] = ws_alloc(3)
            emit_scores_half(3, st3, 0)
            # sm(2) emitted after s3.h0 so the h0 psum evacs (ACT) are not
            # queued behind sm(2)'s exp in the ACT FIFO
            ws_all[2] = ws_alloc(2)
            emit_softmax(2, st2, ws_all[2])
            emit_softmax(3, st3, ws_all[3], cs=slice(0, P))
            emit_scores_half(3, st3, 1)
            emit_softmax(3, st3, ws_all[3], cs=slice(P, S))
            emit_out(1, ws_all[1])
            load_nat(3)
            emit_out(2, ws_all[2])
            emit_out(3, ws_all[3])

    nc.compile()
    return nc


def _get_nc(reps: int = 1, cfg: dict | None = None):
    key = f"nc{reps}-{sorted((cfg or {}).items())}"
    if key not in _CACHE:
        _CACHE[key] = _build_bass(reps, cfg)
    return _CACHE[key]


def _prep(inputs):
    opts = [np.asarray(inputs[f"option{i + 1}"], dtype=np.float32)
            for i in range(NOPT)]
    nat = [np.ascontiguousarray(o.astype(np.float16)) for o in opts]
    xt = [np.ascontiguousarray(o.transpose(0, 2, 1).astype(np.float16))
          for o in opts]
    W = np.ascontiguousarray(np.asarray(inputs["W"], np.float32).astype(np.float16))
    return nat, xt, W


def kernel(**inputs) -> np.ndarray:
    from concourse.bass_utils import run_bass_kernel_spmd

    nc = _get_nc()
    nat, xt, W = _prep(inputs)

    in_maps = []
    for c in range(NCORES):
        m = {}
        for i in range(NOPT):
            m[f"xt{i + 1}"] = xt[i][c * BPC:(c + 1) * BPC]
            m[f"nat{i + 1}"] = nat[i][c * BPC:(c + 1) * BPC]
        m["W"] = W
        in_maps.append(m)

    res = run_bass_kernel_spmd(nc, in_maps, list(range(NCORES)))
    out = np.concatenate([res.results[c]["out"] for c in range(NCORES)], axis=0)
    return np.asarray(out, dtype=np.float32)
